# revision 42
# baseline (speedup 1.0000x reference)
"""GCNConv + GraphNorm + ReLU on 8 Trainium2 NeuronCores.

Strategy (graph-level data parallelism per the natural sharding):
  - Nodes are sharded by graph: core c owns graphs [16c, 16c+16) and their
    nodes (contiguous since batch_ptr is sorted). GraphNorm is then fully
    core-local.
  - Each core computes the full h = X @ W.T table (bf16) into its own HBM
    (replicated compute, ~21us of PE time), because its edges gather from
    arbitrary global src nodes.
  - Edges are bucketed by dst core on the host; per-edge normalization
    coefficients (symmetric GCN norm, incl. self loops) are computed on the
    host and folded into per-128-edge-block one-hot matrices.
  - Per core: bulk dma_gather of h[src] rows (edge-on-partition layout),
    then one matmul per 128-edge block scatters messages into a 128-column
    dst window of PSUM: out[h, c] += sum_e msg[e, h] * onehot[e, c], where
    onehot[e, col(dst_e)] = norm_e. Accumulated in fp32.
  - dma_gather indices are int16, so the h table is addressed in two halves
    (rows < 32768 and >= 32768); host groups each core's edges by half.
  - GraphNorm + ReLU run in the transposed [hidden, node-col] layout with
    per-graph fixed column slots; host un-shards/transposes at the end.
"""

import os
import sys

sys.path.insert(0, "/opt/trn_rl_repo")

import numpy as np
import ml_dtypes

import concourse.bass as bass
import concourse.bacc as bacc
import concourse.tile as tile
import concourse.mybir as mybir

BF16 = ml_dtypes.bfloat16

N_NODES = 50000
N_EDGES = 625000
HID = 128
NUM_GRAPHS = 128
NCORE = 8
GPC = NUM_GRAPHS // NCORE  # graphs per core
HALF = 32768  # int16 gather index limit
GCHUNK = 8192  # max edge slots per dma_gather call
EPS = 1e-5
P = 128


def _ceil(a, b):
    return -(-a // b)


class Meta:
    pass


def prep(node, edge_index, edge_attr, batch_ptr, W, b, gn_weight, gn_bias,
         gn_mean_scale, n_nodes=N_NODES, num_graphs=NUM_GRAPHS):
    """Host-side sharding/layout prep. Returns Meta with per-core arrays."""
    m = Meta()
    N = n_nodes
    NG = num_graphs
    src = np.asarray(edge_index[0]).astype(np.int64)
    dst = np.asarray(edge_index[1]).astype(np.int64)
    ew = np.asarray(edge_attr).astype(np.float32)
    batch = np.asarray(batch_ptr).astype(np.int64)
    node = np.asarray(node).astype(np.float32)
    W = np.asarray(W).astype(np.float32)
    b = np.asarray(b).astype(np.float32)
    gw = np.asarray(gn_weight).astype(np.float32)
    gb = np.asarray(gn_bias).astype(np.float32)
    gms = np.asarray(gn_mean_scale).astype(np.float32)

    # symmetric degree normalization with self loops (f32 like the reference)
    deg = (np.bincount(dst, weights=ew.astype(np.float64), minlength=N)
           .astype(np.float32) + np.float32(1.0))
    dinv = (1.0 / np.sqrt(deg)).astype(np.float32)
    esrc = np.concatenate([src, np.arange(N, dtype=np.int64)])
    edst = np.concatenate([dst, np.arange(N, dtype=np.int64)])
    enorm = np.concatenate([dinv[src] * ew * dinv[dst], dinv * dinv]).astype(np.float32)

    # graph layout: assign graphs to (core, position) so that the 8 graphs
    # sharing a position have similar edge counts (balances the SPMD
    # per-window block capacities), with per-position column slot widths.
    gstart = np.searchsorted(batch, np.arange(NG))
    gend = np.searchsorted(batch, np.arange(NG), side="right")
    cnt = (gend - gstart).astype(np.int64)
    g_of = batch
    gedges = np.bincount(g_of[dst], minlength=NG) + cnt  # in-edges + self loops
    rank = np.argsort(-gedges, kind="stable")
    coreof_g = np.empty(NG, np.int64)
    posof_g = np.empty(NG, np.int64)
    graph_at = np.empty((NCORE, GPC), np.int64)
    gslot_p = np.empty(GPC, np.int64)
    for p_ in range(GPC):
        grp = rank[NCORE * p_: NCORE * (p_ + 1)]
        order = grp if p_ % 2 == 0 else grp[::-1]
        for c_ in range(NCORE):
            coreof_g[order[c_]] = c_
            posof_g[order[c_]] = p_
            graph_at[c_, p_] = order[c_]
        gslot_p[p_] = _ceil(max(int(cnt[grp].max()), 1), 8) * 8
    coff = np.concatenate([[0], np.cumsum(gslot_p)]).astype(np.int64)
    NCOLR = int(coff[-1])
    NWIN = _ceil(NCOLR, P)
    core_of_node = coreof_g[g_of]
    col_of_node = coff[posof_g[g_of]] + (np.arange(N) - gstart[g_of])

    ecore = core_of_node[edst]
    ecol = col_of_node[edst]
    ewin = ecol // P
    ecolm = (ecol % P).astype(np.int64)
    ehalf = (esrc >= HALF).astype(np.int64)

    # per (core, win, half) counts -> shared per-window block capacities
    key = (ecore * NWIN + ewin) * 2 + ehalf
    counts = np.bincount(key, minlength=NCORE * NWIN * 2).reshape(NCORE, NWIN, 2)
    B0 = np.maximum(_ceil(counts[:, :, 0].max(axis=0), P), 1).astype(np.int64)
    B1 = np.maximum(_ceil(counts[:, :, 1].max(axis=0), P), 1).astype(np.int64)
    blk_off0 = np.concatenate([[0], np.cumsum(B0)])
    blk_off1 = np.concatenate([[0], np.cumsum(B1)])
    NSLOT0 = int(blk_off0[-1]) * P
    NSLOT1 = int(blk_off1[-1]) * P
    NSLOT0P = max(NSLOT0, P)
    NSLOT1P = max(NSLOT1, P)
    NSLOTT = NSLOT0P + NSLOT1P
    NBLKT = NSLOTT // P

    # slot assignment: stable-sort edges by (core, half, win); position in group
    order = np.lexsort((ewin, ehalf, ecore))
    key2 = (ecore * 2 + ehalf) * NWIN + ewin
    k_sorted = key2[order]
    grp_change = np.concatenate([[True], k_sorted[1:] != k_sorted[:-1]])
    grp_first = np.where(grp_change)[0]
    grp_id = np.cumsum(grp_change) - 1
    pos_sorted = np.arange(len(order)) - grp_first[grp_id]
    pos = np.empty_like(pos_sorted)
    pos[order] = pos_sorted
    base0 = (blk_off0[:-1] * P)  # [NWIN]
    base1 = NSLOT0P + (blk_off1[:-1] * P)
    ebase = np.where(ehalf == 0, base0[ewin], base1[ewin])
    eslot = ebase + pos  # local slot within the owning core's slot array

    # per-core device arrays
    lidx = np.where(ehalf == 0, esrc, esrc - HALF).astype(np.int16)
    idx_arr = np.zeros((NCORE, NSLOTT), np.int16)
    idx_arr[ecore, eslot] = lidx
    # per-slot one-hot metadata (built into [128, W] blocks on device):
    # meta[c, slot%128, slot//128, 0] = col-within-window, [..1] = norm
    meta = np.zeros((NCORE, P, NBLKT, 2), dtype=np.float32)
    meta[ecore, eslot % P, eslot // P, 0] = ecolm
    meta[ecore, eslot % P, eslot // P, 1] = enorm
    iota = np.ascontiguousarray(
        np.broadcast_to(np.arange(P, dtype=np.float32), (P, P))).astype(BF16)
    idx_wrap = idx_arr.reshape(NCORE, NSLOTT // 16, 16).transpose(0, 2, 1)
    idx_wrap = np.ascontiguousarray(np.tile(idx_wrap, (1, 8, 1)))  # [NCORE,128,S/16]

    # gather table: raw X rows (W is applied after scatter, by linearity)
    NPAD = _ceil(N, P) * P
    xrow = np.zeros((NPAD, HID), dtype=BF16)
    xrow[:N, :] = node.astype(BF16)
    wr = np.ascontiguousarray(W.T).astype(BF16)  # wr[k, j] = W[j, k]

    # GraphNorm constants
    cnt_core = cnt[graph_at].astype(np.float32)  # [NCORE, GPC]
    invc = (1.0 / np.maximum(cnt_core, 1.0)).astype(np.float32)
    invc_t = np.ascontiguousarray(
        np.broadcast_to(invc[:, None, :], (NCORE, P, GPC))).astype(np.float32)
    gnp = np.stack([gms, b * (1.0 - gms), gw, gb], axis=1).astype(np.float32)  # [128,4]

    # device block schedule: per global block: (half, win, first, last) or None
    sched = []
    for w in range(NWIN):
        for i in range(int(B0[w])):
            sched.append((0, w, i == 0, i == int(B0[w]) - 1))
    sched += [None] * ((NSLOT0P - NSLOT0) // P)
    for w in range(NWIN):
        for i in range(int(B1[w])):
            sched.append((1, w, i == 0, i == int(B1[w]) - 1))
    sched += [None] * ((NSLOT1P - NSLOT1) // P)
    assert len(sched) == NBLKT

    m.N, m.NG, m.NPAD = N, NG, NPAD
    m.NCOLR, m.NWIN = NCOLR, NWIN
    m.gslot_p, m.coff, m.graph_at = gslot_p, coff, graph_at
    m.NSLOT0P, m.NSLOT1P, m.NSLOTT, m.NBLKT = NSLOT0P, NSLOT1P, NSLOTT, NBLKT
    m.sched = sched
    m.xrow, m.wr, m.idx_wrap, m.meta, m.iota = xrow, wr, idx_wrap, meta, iota
    m.invc_t, m.gnp = invc_t, gnp
    m.gstart, m.cnt, m.col_of_node, m.core_of_node = gstart, cnt, col_of_node, core_of_node
    return m


def build_nc(m, loop_r=1, phases="abcm"):
    """Build the per-core Bass program (SPMD: same NEFF on all 8 cores).

    loop_r > 1 wraps the whole body in an on-device For loop (timing only).
    """
    import contextlib
    nc = bacc.Bacc("TRN2", target_bir_lowering=False, debug=False)
    dt = mybir.dt

    xrow_d = nc.dram_tensor("xrow", [m.NPAD, HID], dt.bfloat16, kind="ExternalInput")
    wr_d = nc.dram_tensor("wr", [HID, HID], dt.bfloat16, kind="ExternalInput")
    idx_d = nc.dram_tensor("idx", [P, m.NSLOTT // 16], dt.int16, kind="ExternalInput")
    meta_d = nc.dram_tensor("meta", [P, m.NBLKT, 2], dt.float32, kind="ExternalInput")
    iota_d = nc.dram_tensor("iota", [P, P], dt.bfloat16, kind="ExternalInput")
    invc_d = nc.dram_tensor("invc", [P, GPC], dt.float32, kind="ExternalInput")
    gnp_d = nc.dram_tensor("gnp", [P, 4], dt.float32, kind="ExternalInput")
    out_d = nc.dram_tensor("outT", [P, m.NCOLR], dt.float32, kind="ExternalOutput")

    with tile.TileContext(nc) as tc:
        with tc.tile_pool(name="const", bufs=1) as cpool, \
             tc.tile_pool(name="msg", bufs=3) as msgp, \
             tc.tile_pool(name="ohp", bufs=8) as ohp, \
             tc.tile_pool(name="wps", bufs=6, space="PSUM") as wpsp, \
             tc.tile_pool(name="fps", bufs=2, space="PSUM") as fpsp, \
             tc.tile_pool(name="acc", bufs=1) as accp, \
             tc.tile_pool(name="stat", bufs=1) as statp, \
             tc.tile_pool(name="sq", bufs=2) as sqp:

            _lp = contextlib.ExitStack()
            if loop_r > 1:
                _lp.enter_context(tc.For_i(0, loop_r, 1))

            wr_t = cpool.tile([HID, HID], dt.bfloat16)
            nc.sync.dma_start(wr_t[:], wr_d[:])
            idx_t = cpool.tile([P, m.NSLOTT // 16], dt.int16)
            nc.sync.dma_start(idx_t[:], idx_d[:])
            meta_t = cpool.tile([P, m.NBLKT, 2], dt.float32)
            nc.sync.dma_start(meta_t[:], meta_d[:])
            iota_t = cpool.tile([P, P], dt.bfloat16)
            nc.sync.dma_start(iota_t[:], iota_d[:])
            invc_t = cpool.tile([P, GPC], dt.float32)
            nc.sync.dma_start(invc_t[:], invc_d[:])
            gnp_t = cpool.tile([P, 4], dt.float32)
            nc.sync.dma_start(gnp_t[:], gnp_d[:])

            WPC = 4  # windows per acc chunk (512 cols)
            nacc = _ceil(m.NWIN, WPC)
            accC = [accp.tile([P, WPC * P], dt.float32, tag=f"accC{k}",
                              name=f"accC{k}")
                    for k in range(nacc)]

            def acc_slice(w):
                return accC[w // WPC][:, (w % WPC) * P:(w % WPC + 1) * P]

            # ---- Phase B: gather X rows + one-hot scatter matmuls ----
            tab0 = xrow_d[0:HALF, :]
            tab1 = xrow_d[HALF:m.NPAD, :]
            CB = GCHUNK // P  # max blocks per gather chunk
            chunks = []  # (slot_off, n_slots, half)
            for h0, (r0, r1) in enumerate(((0, m.NSLOT0P),
                                           (m.NSLOT0P, m.NSLOTT))):
                off = r0
                while off < r1:
                    n = min(GCHUNK, r1 - off)
                    chunks.append((off, n, h0))
                    off += n
            psum_t = None
            for (s_off, s_n, in_half1) in chunks if "b" in phases else []:
                tab = tab1 if in_half1 else tab0
                msg_t = msgp.tile([P, CB, P], dt.bfloat16, tag="msg")
                nc.gpsimd.dma_gather(
                    msg_t[:, :s_n // P, :], tab[:],
                    idx_t[:, s_off // 16:(s_off + s_n) // 16],
                    num_idxs=s_n, num_idxs_reg=s_n,
                    elem_size=HID, elem_step=HID, single_packet=False)
                for j in range(s_n // P):
                    ent = m.sched[(s_off + j * P) // P]
                    if ent is None or "m" not in phases:
                        continue
                    half, w, first, last = ent
                    blk = (s_off + j * P) // P
                    oh_t = ohp.tile([P, P], dt.bfloat16, tag="oh")
                    nc.vector.tensor_scalar(
                        oh_t[:], iota_t[:],
                        meta_t[:, blk, 0:1], meta_t[:, blk, 1:2],
                        op0=mybir.AluOpType.is_equal, op1=mybir.AluOpType.mult)
                    if first:
                        psum_t = wpsp.tile([P, P], dt.float32, tag="wp", space="PSUM")
                    nc.tensor.matmul(
                        psum_t[:], lhsT=msg_t[:, j, :], rhs=oh_t[:],
                        start=first, stop=last)
                    if last:
                        if half == 0:
                            nc.scalar.activation(
                                acc_slice(w), psum_t[:],
                                mybir.ActivationFunctionType.Copy)
                        else:
                            nc.vector.tensor_tensor(
                                out=acc_slice(w), in0=acc_slice(w),
                                in1=psum_t[:], op=mybir.AluOpType.add)

            # ---- Phase W: conv output = W @ accX (apply W once, by linearity) ----
            convT = accp.tile([P, m.NCOLR], dt.float32, tag="convT")
            if "m" not in phases:
                for k in range(nacc):
                    nc.vector.memset(accC[k][:], 0.0)
            if "a" in phases:
                FCH = WPC * P
                for k in range(nacc):
                    c0 = k * FCH
                    cw = min(FCH, m.NCOLR - c0)
                    vw = min(FCH, m.NWIN * P - c0)  # drained (initialized) cols
                    accB = accp.tile([P, FCH], dt.bfloat16, tag="accB")
                    nc.vector.tensor_copy(accB[:, :vw], accC[k][:, :vw])
                    fp_t = fpsp.tile([P, FCH], dt.float32, tag="fp", space="PSUM")
                    nc.tensor.matmul(fp_t[:, :cw], lhsT=wr_t[:],
                                     rhs=accB[:, :cw],
                                     start=True, stop=True)
                    nc.scalar.activation(convT[:, c0:c0 + cw], fp_t[:, :cw],
                                         mybir.ActivationFunctionType.Copy)
            else:
                for k in range(nacc):
                    c0 = k * FCH if False else k * WPC * P
                    cw = min(WPC * P, m.NCOLR - c0)
                    nc.vector.tensor_copy(convT[:, c0:c0 + cw], accC[k][:, :cw])

            # ---- Phase C: GraphNorm + ReLU (transposed layout) ----
            if "c" in phases:
                ms_ap = gnp_t[:, 0:1]
                cb_ap = gnp_t[:, 1:2]
                gw_ap = gnp_t[:, 2:3]
                gb_ap = gnp_t[:, 3:4]
                st = statp.tile([P, GPC * 8], dt.float32)
                sums = st[:, 0 * GPC:1 * GPC]
                q1 = st[:, 1 * GPC:2 * GPC]
                mu = st[:, 2 * GPC:3 * GPC]
                dd = st[:, 3 * GPC:4 * GPC]
                t1 = st[:, 4 * GPC:5 * GPC]
                var = st[:, 5 * GPC:6 * GPC]
                istd = st[:, 6 * GPC:7 * GPC]
                sh = st[:, 7 * GPC:8 * GPC]

                for g in range(GPC):
                    lo, hi = int(m.coff[g]), int(m.coff[g + 1])
                    nc.vector.tensor_reduce(sums[:, g:g + 1], convT[:, lo:hi],
                                            axis=mybir.AxisListType.X,
                                            op=mybir.AluOpType.add)
                    sq_t = sqp.tile([P, int(m.gslot_p.max())], dt.float32,
                                    tag="sq")
                    nc.scalar.activation(
                        sq_t[:, :hi - lo], convT[:, lo:hi],
                        mybir.ActivationFunctionType.Square,
                        accum_out=q1[:, g:g + 1])
                # mu = sums * invc ; q = q1 * invc (q reuses q1)
                nc.vector.tensor_tensor(mu, sums, invc_t[:], op=mybir.AluOpType.mult)
                nc.vector.tensor_tensor(q1, q1, invc_t[:], op=mybir.AluOpType.mult)
                # d = ms*mu - cb
                nc.vector.tensor_scalar(dd, mu, ms_ap, cb_ap,
                                        op0=mybir.AluOpType.mult,
                                        op1=mybir.AluOpType.subtract)
                # t1 = 2*mu - d ; var = q - d*t1
                nc.vector.tensor_scalar(t1, mu, 2.0, None, op0=mybir.AluOpType.mult)
                nc.vector.tensor_tensor(t1, t1, dd, op=mybir.AluOpType.subtract)
                nc.vector.tensor_tensor(t1, t1, dd, op=mybir.AluOpType.mult)
                nc.vector.tensor_tensor(var, q1, t1, op=mybir.AluOpType.subtract)
                # istd = 1/sqrt(var+eps)
                eps_t = statp.tile([P, 1], dt.float32, tag="eps")
                nc.vector.memset(eps_t[:], float(EPS))
                nc.scalar.activation(istd, var, mybir.ActivationFunctionType.Sqrt,
                                     bias=eps_t[:])
                nc.vector.reciprocal(istd, istd)
                # scale = gw*istd (into istd); sh = gb - scale*d
                nc.vector.tensor_scalar(istd, istd, gw_ap, None,
                                        op0=mybir.AluOpType.mult)
                nc.vector.tensor_tensor(sh, istd, dd, op=mybir.AluOpType.mult)
                nc.vector.tensor_scalar(sh, sh, -1.0, gb_ap,
                                        op0=mybir.AluOpType.mult,
                                        op1=mybir.AluOpType.add)
                outT = accp.tile([P, m.NCOLR], dt.float32)
                for g in range(GPC):
                    nc.scalar.activation(
                        outT[:, int(m.coff[g]):int(m.coff[g + 1])],
                        convT[:, int(m.coff[g]):int(m.coff[g + 1])],
                        mybir.ActivationFunctionType.Relu,
                        bias=sh[:, g:g + 1], scale=istd[:, g:g + 1])
                nc.sync.dma_start(out_d[:], outT[:])
            else:
                nc.sync.dma_start(out_d[:], convT[:])
            _lp.close()

    nc.compile()
    return nc


def in_maps_for(m):
    maps = []
    for c in range(NCORE):
        maps.append({
            "xrow": m.xrow,
            "wr": m.wr,
            "idx": m.idx_wrap[c],
            "meta": m.meta[c],
            "iota": m.iota,
            "invc": m.invc_t[c],
            "gnp": m.gnp,
        })
    return maps


def unshard(m, outs):
    """outs: list of per-core {'outT': [128, NCOLR]} -> full [N, 128] f32."""
    res = np.empty((m.N, HID), dtype=np.float32)
    for c in range(NCORE):
        oT = outs[c]["outT"]
        for gl in range(GPC):
            g = int(m.graph_at[c, gl])
            n0 = int(m.gstart[g])
            k = int(m.cnt[g])
            if k:
                lo = int(m.coff[gl])
                res[n0:n0 + k, :] = oT[:, lo:lo + k].T
    return res


def kernel(node, edge_index, edge_attr, batch_ptr, W, b, gn_weight, gn_bias,
           gn_mean_scale):
    from concourse import bass_utils
    m = prep(node, edge_index, edge_attr, batch_ptr, W, b, gn_weight, gn_bias,
             gn_mean_scale)
    nc = build_nc(m)
    res = bass_utils.run_bass_kernel_spmd(nc, in_maps_for(m),
                                          core_ids=list(range(NCORE)))
    return unshard(m, res.results)


# revision 46
# speedup vs baseline: 1.4814x; 1.4814x over previous
"""GCNConv + GraphNorm + ReLU on 8 Trainium2 NeuronCores.

Strategy (graph-level data parallelism per the natural sharding):
  - Nodes are sharded by graph: each core owns 16 graphs; graphs are
    assigned to (core, position) so the 8 graphs sharing a position have
    similar edge counts (balances SPMD capacities). GraphNorm is fully
    core-local.
  - By linearity, sum_e norm_e*(X@W.T)[src_e] = (sum_e norm_e*X[src_e])@W.T:
    cores scatter-accumulate raw X rows and apply W once to the result.
  - Edges are bucketed by dst core on the host; symmetric GCN normalization
    (incl. self loops) is computed on the host into per-slot (col, norm)
    metadata.
  - Per core: bulk dma_gather of X[src] rows (bf16, edge-on-partition
    layout); per 128-edge block the DVE builds a one-hot [128,128] with
    norm folded in, and one PE matmul scatters the block into a 128-column
    dst window of PSUM: acc[x, c] += sum_e X[src_e, x] * onehot[e, c].
    Accumulated fp32 in SBUF chunks, then conv = W @ acc via one stationary
    matmul pass.
  - dma_gather indices are int16, so the X table is addressed in two halves
    (rows < 32768 and >= 32768); host groups each core's edges by half.
  - GraphNorm + ReLU run in the transposed [hidden, node-col] layout with
    per-graph fixed column slots; host un-shards/transposes at the end.
"""

import os
import sys

sys.path.insert(0, "/opt/trn_rl_repo")

import numpy as np
import ml_dtypes

import concourse.bass as bass
import concourse.bacc as bacc
import concourse.tile as tile
import concourse.mybir as mybir

BF16 = ml_dtypes.bfloat16

N_NODES = 50000
N_EDGES = 625000
HID = 128
NUM_GRAPHS = 128
NCORE = 8
GPC = NUM_GRAPHS // NCORE  # graphs per core
HALF = 32768  # int16 gather index limit
GCHUNK = 8192  # max edge slots per dma_gather call
# "ant" = dma_gather (int16 halves). "ind" (indirect DMA, int32) is kept for
# reference but is NOT HW-correct: the ucode honors only one offset per
# partition (CoreSim accepts [P, K] offsets; hardware does not).
GATHER_MODE = "ant"
EPS = 1e-5
P = 128


def _ceil(a, b):
    return -(-a // b)


class Meta:
    pass


def prep(node, edge_index, edge_attr, batch_ptr, W, b, gn_weight, gn_bias,
         gn_mean_scale, n_nodes=N_NODES, num_graphs=NUM_GRAPHS):
    """Host-side sharding/layout prep. Returns Meta with per-core arrays."""
    m = Meta()
    N = n_nodes
    NG = num_graphs
    src = np.asarray(edge_index[0]).astype(np.int64)
    dst = np.asarray(edge_index[1]).astype(np.int64)
    ew = np.asarray(edge_attr).astype(np.float32)
    batch = np.asarray(batch_ptr).astype(np.int64)
    node = np.asarray(node).astype(np.float32)
    W = np.asarray(W).astype(np.float32)
    b = np.asarray(b).astype(np.float32)
    gw = np.asarray(gn_weight).astype(np.float32)
    gb = np.asarray(gn_bias).astype(np.float32)
    gms = np.asarray(gn_mean_scale).astype(np.float32)

    # symmetric degree normalization with self loops (f32 like the reference)
    deg = (np.bincount(dst, weights=ew.astype(np.float64), minlength=N)
           .astype(np.float32) + np.float32(1.0))
    dinv = (1.0 / np.sqrt(deg)).astype(np.float32)
    esrc = np.concatenate([src, np.arange(N, dtype=np.int64)])
    edst = np.concatenate([dst, np.arange(N, dtype=np.int64)])
    enorm = np.concatenate([dinv[src] * ew * dinv[dst], dinv * dinv]).astype(np.float32)

    # graph layout: assign graphs to (core, position) so that the 8 graphs
    # sharing a position have similar edge counts (balances the SPMD
    # per-window block capacities), with per-position column slot widths.
    gstart = np.searchsorted(batch, np.arange(NG))
    gend = np.searchsorted(batch, np.arange(NG), side="right")
    cnt = (gend - gstart).astype(np.int64)
    g_of = batch
    gedges = np.bincount(g_of[dst], minlength=NG) + cnt  # in-edges + self loops
    rank = np.argsort(-gedges, kind="stable")
    coreof_g = np.empty(NG, np.int64)
    posof_g = np.empty(NG, np.int64)
    graph_at = np.empty((NCORE, GPC), np.int64)
    gslot_p = np.empty(GPC, np.int64)
    for p_ in range(GPC):
        grp = rank[NCORE * p_: NCORE * (p_ + 1)]
        order = grp if p_ % 2 == 0 else grp[::-1]
        for c_ in range(NCORE):
            coreof_g[order[c_]] = c_
            posof_g[order[c_]] = p_
            graph_at[c_, p_] = order[c_]
        gslot_p[p_] = _ceil(max(int(cnt[grp].max()), 1), 8) * 8
    coff = np.concatenate([[0], np.cumsum(gslot_p)]).astype(np.int64)
    NCOLR = int(coff[-1])
    NWIN = _ceil(NCOLR, P)
    core_of_node = coreof_g[g_of]
    col_of_node = coff[posof_g[g_of]] + (np.arange(N) - gstart[g_of])

    ecore = core_of_node[edst]
    ecol = col_of_node[edst]
    ewin = ecol // P
    ecolm = (ecol % P).astype(np.int64)
    ehalf = (esrc >= HALF).astype(np.int64)

    # per (core, win, half) counts -> shared per-window block capacities
    key = (ecore * NWIN + ewin) * 2 + ehalf
    counts = np.bincount(key, minlength=NCORE * NWIN * 2).reshape(NCORE, NWIN, 2)
    B0 = np.maximum(_ceil(counts[:, :, 0].max(axis=0), P), 1).astype(np.int64)
    B1 = np.maximum(_ceil(counts[:, :, 1].max(axis=0), P), 1).astype(np.int64)
    blk_off0 = np.concatenate([[0], np.cumsum(B0)])
    blk_off1 = np.concatenate([[0], np.cumsum(B1)])
    NSLOT0 = int(blk_off0[-1]) * P
    NSLOT1 = int(blk_off1[-1]) * P
    NSLOT0P = max(NSLOT0, P)
    NSLOT1P = max(NSLOT1, P)
    NSLOTT = NSLOT0P + NSLOT1P
    NBLKT = NSLOTT // P

    # slot assignment: stable-sort edges by (core, half, win); position in group
    order = np.lexsort((ewin, ehalf, ecore))
    key2 = (ecore * 2 + ehalf) * NWIN + ewin
    k_sorted = key2[order]
    grp_change = np.concatenate([[True], k_sorted[1:] != k_sorted[:-1]])
    grp_first = np.where(grp_change)[0]
    grp_id = np.cumsum(grp_change) - 1
    pos_sorted = np.arange(len(order)) - grp_first[grp_id]
    pos = np.empty_like(pos_sorted)
    pos[order] = pos_sorted
    base0 = (blk_off0[:-1] * P)  # [NWIN]
    base1 = NSLOT0P + (blk_off1[:-1] * P)
    ebase = np.where(ehalf == 0, base0[ewin], base1[ewin])
    eslot = ebase + pos  # local slot within the owning core's slot array

    # per-core device arrays
    lidx = np.where(ehalf == 0, esrc, esrc - HALF).astype(np.int16)
    idx_arr = np.zeros((NCORE, NSLOTT), np.int16)
    idx_arr[ecore, eslot] = lidx
    # per-slot one-hot metadata (built into [128, W] blocks on device):
    # meta[c, slot%128, slot//128, 0] = col-within-window, [..1] = norm
    meta = np.zeros((NCORE, P, NBLKT, 2), dtype=np.float32)
    meta[ecore, eslot % P, eslot // P, 0] = ecolm
    meta[ecore, eslot % P, eslot // P, 1] = enorm
    iota = np.ascontiguousarray(
        np.broadcast_to(np.arange(P, dtype=np.float32), (P, P))).astype(BF16)
    idx_wrap = idx_arr.reshape(NCORE, NSLOTT // 16, 16).transpose(0, 2, 1)
    idx_wrap = np.ascontiguousarray(np.tile(idx_wrap, (1, 8, 1)))  # [NCORE,128,S/16]
    # int32 global offsets for the indirect-DMA gather path: off[c, s%128, s//128]
    off_arr = np.zeros((NCORE, P, NSLOTT // P), np.int32)
    off_arr[ecore, eslot % P, eslot // P] = esrc.astype(np.int32)

    # gather table: raw X rows (W is applied after scatter, by linearity)
    NPAD = _ceil(N, P) * P
    xrow = np.zeros((NPAD, HID), dtype=BF16)
    xrow[:N, :] = node.astype(BF16)
    wr = np.ascontiguousarray(W.T).astype(BF16)  # wr[k, j] = W[j, k]

    # GraphNorm constants
    cnt_core = cnt[graph_at].astype(np.float32)  # [NCORE, GPC]
    invc = (1.0 / np.maximum(cnt_core, 1.0)).astype(np.float32)
    invc_t = np.ascontiguousarray(
        np.broadcast_to(invc[:, None, :], (NCORE, P, GPC))).astype(np.float32)
    gnp = np.stack([gms, b * (1.0 - gms), gw, gb], axis=1).astype(np.float32)  # [128,4]

    # device block schedule: per global block: (half, win, first, last) or None
    sched = []
    for w in range(NWIN):
        for i in range(int(B0[w])):
            sched.append((0, w, i == 0, i == int(B0[w]) - 1))
    sched += [None] * ((NSLOT0P - NSLOT0) // P)
    for w in range(NWIN):
        for i in range(int(B1[w])):
            sched.append((1, w, i == 0, i == int(B1[w]) - 1))
    sched += [None] * ((NSLOT1P - NSLOT1) // P)
    assert len(sched) == NBLKT

    m.N, m.NG, m.NPAD = N, NG, NPAD
    m.NCOLR, m.NWIN = NCOLR, NWIN
    m.gslot_p, m.coff, m.graph_at = gslot_p, coff, graph_at
    m.NSLOT0P, m.NSLOT1P, m.NSLOTT, m.NBLKT = NSLOT0P, NSLOT1P, NSLOTT, NBLKT
    m.sched = sched
    m.xrow, m.wr, m.idx_wrap, m.meta, m.iota = xrow, wr, idx_wrap, meta, iota
    m.off_arr = off_arr
    m.invc_t, m.gnp = invc_t, gnp
    m.gstart, m.cnt, m.col_of_node, m.core_of_node = gstart, cnt, col_of_node, core_of_node
    return m


def build_nc(m, loop_r=1, phases="abcm"):
    """Build the per-core Bass program (SPMD: same NEFF on all 8 cores).

    loop_r > 1 wraps the whole body in an on-device For loop (timing only).
    """
    import contextlib
    nc = bacc.Bacc("TRN2", target_bir_lowering=False, debug=False)
    dt = mybir.dt

    xrow_d = nc.dram_tensor("xrow", [m.NPAD, HID], dt.bfloat16, kind="ExternalInput")
    wr_d = nc.dram_tensor("wr", [HID, HID], dt.bfloat16, kind="ExternalInput")
    if GATHER_MODE == "ind":
        idx_d = nc.dram_tensor("idx", [P, m.NSLOTT // P], dt.int32,
                               kind="ExternalInput")
    else:
        idx_d = nc.dram_tensor("idx", [P, m.NSLOTT // 16], dt.int16,
                               kind="ExternalInput")
    meta_d = nc.dram_tensor("meta", [P, m.NBLKT, 2], dt.float32, kind="ExternalInput")
    iota_d = nc.dram_tensor("iota", [P, P], dt.bfloat16, kind="ExternalInput")
    invc_d = nc.dram_tensor("invc", [P, GPC], dt.float32, kind="ExternalInput")
    gnp_d = nc.dram_tensor("gnp", [P, 4], dt.float32, kind="ExternalInput")
    out_d = nc.dram_tensor("outT", [P, m.NCOLR], dt.float32, kind="ExternalOutput")

    with tile.TileContext(nc) as tc:
        with tc.tile_pool(name="const", bufs=1) as cpool, \
             tc.tile_pool(name="msg", bufs=3) as msgp, \
             tc.tile_pool(name="ohp", bufs=8) as ohp, \
             tc.tile_pool(name="wps", bufs=6, space="PSUM") as wpsp, \
             tc.tile_pool(name="fps", bufs=2, space="PSUM") as fpsp, \
             tc.tile_pool(name="acc", bufs=1) as accp, \
             tc.tile_pool(name="stat", bufs=1) as statp, \
             tc.tile_pool(name="sq", bufs=2) as sqp:

            _lp = contextlib.ExitStack()
            if loop_r > 1:
                _lp.enter_context(tc.For_i(0, loop_r, 1))

            wr_t = cpool.tile([HID, HID], dt.bfloat16)
            nc.sync.dma_start(wr_t[:], wr_d[:])
            if GATHER_MODE == "ind":
                idx_t = cpool.tile([P, m.NSLOTT // P], dt.int32)
            else:
                idx_t = cpool.tile([P, m.NSLOTT // 16], dt.int16)
            nc.sync.dma_start(idx_t[:], idx_d[:])
            meta_t = cpool.tile([P, m.NBLKT, 2], dt.float32)
            nc.sync.dma_start(meta_t[:], meta_d[:])
            iota_t = cpool.tile([P, P], dt.bfloat16)
            nc.sync.dma_start(iota_t[:], iota_d[:])
            invc_t = cpool.tile([P, GPC], dt.float32)
            nc.sync.dma_start(invc_t[:], invc_d[:])
            gnp_t = cpool.tile([P, 4], dt.float32)
            nc.sync.dma_start(gnp_t[:], gnp_d[:])

            WPC = 4  # windows per acc chunk (512 cols)
            nacc = _ceil(m.NWIN, WPC)
            accC = [accp.tile([P, WPC * P], dt.float32, tag=f"accC{k}",
                              name=f"accC{k}")
                    for k in range(nacc)]

            def acc_slice(w):
                return accC[w // WPC][:, (w % WPC) * P:(w % WPC + 1) * P]

            # ---- Phase B: gather X rows + one-hot scatter matmuls ----
            tab0 = xrow_d[0:HALF, :]
            tab1 = xrow_d[HALF:m.NPAD, :]
            CB = GCHUNK // P  # max blocks per gather chunk
            chunks = []  # (slot_off, n_slots, half)
            for h0, (r0, r1) in enumerate(((0, m.NSLOT0P),
                                           (m.NSLOT0P, m.NSLOTT))):
                off = r0
                while off < r1:
                    n = min(GCHUNK, r1 - off)
                    chunks.append((off, n, h0))
                    off += n
            psum_t = None
            for (s_off, s_n, in_half1) in chunks if "b" in phases else []:
                tab = tab1 if in_half1 else tab0
                msg_t = msgp.tile([P, CB, P], dt.bfloat16, tag="msg")
                if GATHER_MODE == "ind":
                    nc.gpsimd.indirect_dma_start(
                        out=msg_t[:, :s_n // P, :], out_offset=None,
                        in_=xrow_d[:],
                        in_offset=bass.IndirectOffsetOnAxis(
                            ap=idx_t[:, s_off // P:(s_off + s_n) // P],
                            axis=0))
                else:
                    nc.gpsimd.dma_gather(
                        msg_t[:, :s_n // P, :], tab[:],
                        idx_t[:, s_off // 16:(s_off + s_n) // 16],
                        num_idxs=s_n, num_idxs_reg=s_n,
                        elem_size=HID, elem_step=HID, single_packet=False)
                for j in range(s_n // P):
                    ent = m.sched[(s_off + j * P) // P]
                    if ent is None or "m" not in phases:
                        continue
                    half, w, first, last = ent
                    blk = (s_off + j * P) // P
                    oh_t = ohp.tile([P, P], dt.bfloat16, tag="oh")
                    nc.vector.tensor_scalar(
                        oh_t[:], iota_t[:],
                        meta_t[:, blk, 0:1], meta_t[:, blk, 1:2],
                        op0=mybir.AluOpType.is_equal, op1=mybir.AluOpType.mult)
                    if first:
                        psum_t = wpsp.tile([P, P], dt.float32, tag="wp", space="PSUM")
                    nc.tensor.matmul(
                        psum_t[:], lhsT=msg_t[:, j, :], rhs=oh_t[:],
                        start=first, stop=last)
                    if last:
                        if half == 0:
                            nc.scalar.activation(
                                acc_slice(w), psum_t[:],
                                mybir.ActivationFunctionType.Copy)
                        else:
                            nc.vector.tensor_tensor(
                                out=acc_slice(w), in0=acc_slice(w),
                                in1=psum_t[:], op=mybir.AluOpType.add)

            # ---- Phase W: conv output = W @ accX (apply W once, by linearity) ----
            convT = accp.tile([P, m.NCOLR], dt.float32, tag="convT")
            if "m" not in phases:
                for k in range(nacc):
                    nc.vector.memset(accC[k][:], 0.0)
            if "a" in phases:
                FCH = WPC * P
                for k in range(nacc):
                    c0 = k * FCH
                    cw = min(FCH, m.NCOLR - c0)
                    vw = min(FCH, m.NWIN * P - c0)  # drained (initialized) cols
                    accB = accp.tile([P, FCH], dt.bfloat16, tag="accB")
                    nc.vector.tensor_copy(accB[:, :vw], accC[k][:, :vw])
                    fp_t = fpsp.tile([P, FCH], dt.float32, tag="fp", space="PSUM")
                    nc.tensor.matmul(fp_t[:, :cw], lhsT=wr_t[:],
                                     rhs=accB[:, :cw],
                                     start=True, stop=True)
                    nc.scalar.activation(convT[:, c0:c0 + cw], fp_t[:, :cw],
                                         mybir.ActivationFunctionType.Copy)
            else:
                for k in range(nacc):
                    c0 = k * FCH if False else k * WPC * P
                    cw = min(WPC * P, m.NCOLR - c0)
                    nc.vector.tensor_copy(convT[:, c0:c0 + cw], accC[k][:, :cw])

            # ---- Phase C: GraphNorm + ReLU (transposed layout) ----
            if "c" in phases:
                ms_ap = gnp_t[:, 0:1]
                cb_ap = gnp_t[:, 1:2]
                gw_ap = gnp_t[:, 2:3]
                gb_ap = gnp_t[:, 3:4]
                st = statp.tile([P, GPC * 8], dt.float32)
                sums = st[:, 0 * GPC:1 * GPC]
                q1 = st[:, 1 * GPC:2 * GPC]
                mu = st[:, 2 * GPC:3 * GPC]
                dd = st[:, 3 * GPC:4 * GPC]
                t1 = st[:, 4 * GPC:5 * GPC]
                var = st[:, 5 * GPC:6 * GPC]
                istd = st[:, 6 * GPC:7 * GPC]
                sh = st[:, 7 * GPC:8 * GPC]

                for g in range(GPC):
                    lo, hi = int(m.coff[g]), int(m.coff[g + 1])
                    nc.vector.tensor_reduce(sums[:, g:g + 1], convT[:, lo:hi],
                                            axis=mybir.AxisListType.X,
                                            op=mybir.AluOpType.add)
                    sq_t = sqp.tile([P, int(m.gslot_p.max())], dt.float32,
                                    tag="sq")
                    nc.scalar.activation(
                        sq_t[:, :hi - lo], convT[:, lo:hi],
                        mybir.ActivationFunctionType.Square,
                        accum_out=q1[:, g:g + 1])
                # mu = sums * invc ; q = q1 * invc (q reuses q1)
                nc.vector.tensor_tensor(mu, sums, invc_t[:], op=mybir.AluOpType.mult)
                nc.vector.tensor_tensor(q1, q1, invc_t[:], op=mybir.AluOpType.mult)
                # d = ms*mu - cb
                nc.vector.tensor_scalar(dd, mu, ms_ap, cb_ap,
                                        op0=mybir.AluOpType.mult,
                                        op1=mybir.AluOpType.subtract)
                # t1 = 2*mu - d ; var = q - d*t1
                nc.vector.tensor_scalar(t1, mu, 2.0, None, op0=mybir.AluOpType.mult)
                nc.vector.tensor_tensor(t1, t1, dd, op=mybir.AluOpType.subtract)
                nc.vector.tensor_tensor(t1, t1, dd, op=mybir.AluOpType.mult)
                nc.vector.tensor_tensor(var, q1, t1, op=mybir.AluOpType.subtract)
                # istd = 1/sqrt(var+eps)
                eps_t = statp.tile([P, 1], dt.float32, tag="eps")
                nc.vector.memset(eps_t[:], float(EPS))
                nc.scalar.activation(istd, var, mybir.ActivationFunctionType.Sqrt,
                                     bias=eps_t[:])
                nc.vector.reciprocal(istd, istd)
                # scale = gw*istd (into istd); sh = gb - scale*d
                nc.vector.tensor_scalar(istd, istd, gw_ap, None,
                                        op0=mybir.AluOpType.mult)
                nc.vector.tensor_tensor(sh, istd, dd, op=mybir.AluOpType.mult)
                nc.vector.tensor_scalar(sh, sh, -1.0, gb_ap,
                                        op0=mybir.AluOpType.mult,
                                        op1=mybir.AluOpType.add)
                outT = accp.tile([P, m.NCOLR], dt.float32)
                for g in range(GPC):
                    nc.scalar.activation(
                        outT[:, int(m.coff[g]):int(m.coff[g + 1])],
                        convT[:, int(m.coff[g]):int(m.coff[g + 1])],
                        mybir.ActivationFunctionType.Relu,
                        bias=sh[:, g:g + 1], scale=istd[:, g:g + 1])
                nc.sync.dma_start(out_d[:], outT[:])
            else:
                nc.sync.dma_start(out_d[:], convT[:])
            _lp.close()

    nc.compile()
    return nc


def in_maps_for(m):
    maps = []
    for c in range(NCORE):
        maps.append({
            "xrow": m.xrow,
            "wr": m.wr,
            "idx": m.off_arr[c] if GATHER_MODE == "ind" else m.idx_wrap[c],
            "meta": m.meta[c],
            "iota": m.iota,
            "invc": m.invc_t[c],
            "gnp": m.gnp,
        })
    return maps


def unshard(m, outs):
    """outs: list of per-core {'outT': [128, NCOLR]} -> full [N, 128] f32."""
    res = np.empty((m.N, HID), dtype=np.float32)
    for c in range(NCORE):
        oT = outs[c]["outT"]
        for gl in range(GPC):
            g = int(m.graph_at[c, gl])
            n0 = int(m.gstart[g])
            k = int(m.cnt[g])
            if k:
                lo = int(m.coff[gl])
                res[n0:n0 + k, :] = oT[:, lo:lo + k].T
    return res


def kernel(node, edge_index, edge_attr, batch_ptr, W, b, gn_weight, gn_bias,
           gn_mean_scale):
    from concourse import bass_utils
    m = prep(node, edge_index, edge_attr, batch_ptr, W, b, gn_weight, gn_bias,
             gn_mean_scale)
    nc = build_nc(m)
    res = bass_utils.run_bass_kernel_spmd(nc, in_maps_for(m),
                                          core_ids=list(range(NCORE)))
    return unshard(m, res.results)
